# revision 38
# baseline (speedup 1.0000x reference)
"""DeepAR single-step LSTM (3 layers, B=1024, H=1024) on 8 trn2 NeuronCores.

Sharding: model-parallel over the hidden-feature axis. Core m owns features
Fm = [128m, 128m+128) of every gate block in every layer, i.e. rows
{g*1024 + Fm : g in 0..3} of each [4096, K] weight. Everything on device is
feature-major ([features, batch]), so no on-device transposes of activations
are ever needed:

  - gates_m.T [512, B] = Wm.T-chunks (lhsT, [K=128,128]) x state.T chunks
    (rhs, [K=128, 512]), accumulated over K in PSUM.
  - Matmul inputs (weights, old/new hidden states) are bf16 (or fp32r);
    PSUM accumulation, the LSTM cell elementwise math, cell state, biases
    and all outputs stay fp32.
  - LSTM cell elementwise on [128, B] feature-major tiles (ACT + DVE).
  - new h slices are AllGather'd per layer so layers 1/2 input matmuls can
    run; the layer-2 w_hh pass is interleaved to cover AG latency.
  - head (mu/softplus-sigma) = tiny PE matmuls + AllReduce of [2, B]
    partials.

Host side only reshapes/transposes/slices inputs (weight layout prep,
dtype casts) and concatenates outputs; all model math runs on device.
"""

import numpy as np
import ml_dtypes

import concourse.bacc as bacc
import concourse.bass as bass
import concourse.mybir as mybir
import concourse.tile as tile
from concourse.bass import IndirectOffsetOnAxis
from concourse.bass_utils import run_bass_kernel_spmd
from concourse.masks import make_identity

P = 128
B = 1024
H = 1024
L = 3
NCORES = 8
EMB = 64
COV = 4
IN_DIM = 1 + COV + EMB  # 69
NUM_CLASSES = 32768
GH = 4 * P  # gate rows per core (512)
NH = 512  # matmul free-dim chunk (batch half)
KH = H // P  # 8 K-chunks over the hidden dim
KHH = KH // 2

f32 = mybir.dt.float32
i32 = mybir.dt.int32
ADD = mybir.AluOpType.add
MULT = mybir.AluOpType.mult
AF = mybir.ActivationFunctionType

# matmul-input dtype: bfloat16 (full PE rate, FWL weight loads) or float32r
# (single-pass fp32, ~2x slower, higher precision)
MM_DT = mybir.dt.bfloat16
MM_NP = ml_dtypes.bfloat16


def build_program() -> bass.Bass:
    nc = bacc.Bacc("TRN2", target_bir_lowering=False, debug=False,
                   num_devices=NCORES)
    RG = [list(range(NCORES))]

    # ------------- DRAM I/O (per-core shapes) -------------
    # pre-shuffled on host so every DMA is per-partition contiguous:
    #   w*(T): [P, KH, GH]  with [p, k, m] = w[m_row, k*128+p]
    #   hiddenT: [L, P, KH, B] with [l, p, k, b] = hidden[l, b, k*128+p]
    xT = nc.dram_tensor("xT", [1 + COV, B], f32, kind="ExternalInput").ap()
    idx32 = nc.dram_tensor("idx32", [P, 8], i32, kind="ExternalInput").ap()
    emb = nc.dram_tensor("emb", [NUM_CLASSES, EMB], f32,
                         kind="ExternalInput").ap()
    hiddenT = nc.dram_tensor("hiddenT", [L, P, KH, B], MM_DT,
                             kind="ExternalInput").ap()
    cellT = nc.dram_tensor("cellT", [L, P, B], f32, kind="ExternalInput").ap()
    wih0T = nc.dram_tensor("wih0T", [P, GH], MM_DT, kind="ExternalInput").ap()
    wihT = nc.dram_tensor("wihT", [L - 1, P, KH, GH], MM_DT,
                          kind="ExternalInput").ap()
    whhT = nc.dram_tensor("whhT", [L, P, KH, GH], MM_DT,
                          kind="ExternalInput").ap()
    biasv = nc.dram_tensor("biasv", [P, 12], f32, kind="ExternalInput").ap()
    whead = nc.dram_tensor("whead", [P, L, 2], f32, kind="ExternalInput").ap()
    bhead = nc.dram_tensor("bhead", [1, 2], f32, kind="ExternalInput").ap()

    hT_out = nc.dram_tensor("hT_out", [L, P, B], f32,
                            kind="ExternalOutput").ap()
    cT_out = nc.dram_tensor("cT_out", [L, P, B], f32,
                            kind="ExternalOutput").ap()
    head_out = nc.dram_tensor("head_out", [2, B], f32,
                              kind="ExternalOutput").ap()

    # ------------- internal DRAM (collective bounce) -------------
    ag_in = [nc.dram_tensor(f"ag_in_{l}", [P, B], MM_DT).ap()
             for l in range(2)]
    ag_out = [nc.dram_tensor(f"ag_out_{l}", [H, B], MM_DT,
                             addr_space="Shared").ap()
              for l in range(2)]
    ar_in = nc.dram_tensor("ar_in", [2, B], f32).ap()
    ar_out = nc.dram_tensor("ar_out", [2, B], f32, addr_space="Shared").ap()

    with tile.TileContext(nc) as tc, \
            tc.tile_pool(name="const", bufs=1) as constp, \
            tc.tile_pool(name="state", bufs=1) as statep, \
            tc.tile_pool(name="wp", bufs=10) as wp, \
            tc.tile_pool(name="rp", bufs=10) as rp, \
            tc.tile_pool(name="ep", bufs=2) as ep, \
            tc.tile_pool(name="psg", bufs=5, space="PSUM") as psg, \
            tc.tile_pool(name="pst", bufs=2, space="PSUM") as pstp, \
            tc.tile_pool(name="psh", bufs=1, space="PSUM") as pshp:

        # ---------- constants / small inputs ----------
        # latency-class loads go on the scalar HWDGE ring; the sync ring is
        # reserved for the big streaming loads so the first matmuls start
        # as early as possible.
        idx_sb = constp.tile([P, 8], i32, tag="idx")
        nc.scalar.dma_start(idx_sb[:], idx32)
        bias_sb = constp.tile([P, 12], f32, tag="bias")
        nc.scalar.dma_start(bias_sb[:], biasv)
        whead_sb = constp.tile([P, L, 2], f32, tag="whead")
        nc.scalar.dma_start(whead_sb[:], whead)
        bh_sb = constp.tile([1, 2], f32, tag="bh")
        nc.scalar.dma_start(bh_sb[:], bhead)
        ident = constp.tile([P, P], f32, tag="ident")
        make_identity(nc, ident[:])
        wih0_sb = constp.tile([P, GH], MM_DT, tag="wih0")
        nc.scalar.dma_start(wih0_sb[:], wih0T)
        # preload ACT function tables off the critical path
        warm = constp.tile([1, 4], f32, tag="actwarm")
        nc.vector.memset(warm[:], 1.0)
        for fn in (AF.Sigmoid, AF.Tanh, AF.Exp, AF.Ln, AF.Identity):
            nc.scalar.activation(warm[:], warm[:], fn)

        # layer-0 input, feature-major [K=128, B]: x at partitions 0:5,
        # emb at partitions 64:128 (engine writes need 32-aligned starts),
        # rest zero; wih0T host layout matches.
        inp0 = constp.tile([P, B], MM_DT, tag="inp0")
        nc.any.memzero(inp0[:])
        nc.gpsimd.dma_start(out=inp0[0:1 + COV, :], in_=xT)  # cast dma

        # one gather per 128-batch tile: HW indirect DMA honors exactly one
        # offset per partition ([P, 1] offset AP)
        emb_raw = constp.tile([P, 8, EMB], f32, tag="embraw")
        for c in range(8):
            nc.gpsimd.indirect_dma_start(
                out=emb_raw[:, c, :], out_offset=None, in_=emb,
                in_offset=IndirectOffsetOnAxis(ap=idx_sb[:, c:c + 1], axis=0))

        # persistent state
        h_sb = [statep.tile([P, B], f32, tag=f"h{l}", name=f"h{l}")
                for l in range(L)]
        acc = {l: statep.tile([P, 4, B], f32, tag=f"acc{l}", name=f"acc{l}")
               for l in (1, 2)}

        def bcol(l, g):
            return bias_sb[:, l * 4 + g:l * 4 + g + 1]

        KQ = 2  # k-chunks per streamed quarter-tile

        def load_w(dram3d):
            # dram3d: [P, KH, GH] -> four [P, KQ, GH] tiles
            qs = []
            for q in range(KH // KQ):
                t = wp.tile([P, KQ, GH], MM_DT, tag="w")
                nc.sync.dma_start(t[:], dram3d[:, q * KQ:(q + 1) * KQ, :])
                qs.append(t)
            return qs

        def load_rhs_hidden(l):
            # hiddenT[l]: [P, KH, B] -> four [P, KQ, B] tiles (full batch)
            qs = []
            for q in range(KH // KQ):
                t = rp.tile([P, KQ, B], MM_DT, tag="r")
                nc.sync.dma_start(
                    t[:], hiddenT[l][:, q * KQ:(q + 1) * KQ, :])
                qs.append(t)
            return qs

        def load_rhs_ag(l):
            # ag_out[l]: [H, B] rank-block rows -> [P, 1, B] tiles; eighth
            # granularity so pass-B starts ~1us after the AG completes
            r = ag_out[l].rearrange("(ko p) b -> p ko b", p=P)
            es = []
            for q in range(KH):
                t = rp.tile([P, 1, B], MM_DT, tag="re")
                nc.sync.dma_start(t[:], r[:, q:q + 1, :])
                es.append(t)
            return es

        def chain8(ps, wqs, r8, g, n, start, stop):
            # chain variant for eighth-granularity rhs tiles
            ns = slice(n * NH, (n + 1) * NH)
            for kk in range(KH):
                nc.tensor.matmul(
                    ps,
                    lhsT=wqs[kk // KQ][:, kk % KQ, g * P:(g + 1) * P],
                    rhs=r8[kk][:, 0, ns],
                    start=start and kk == 0, stop=stop and kk == KH - 1)

        def chain(ps, wqs, rqs, g, n, start, stop):
            ns = slice(n * NH, (n + 1) * NH)
            for q in range(KH // KQ):
                wt, rt = wqs[q], rqs[q]
                for k in range(KQ):
                    first = start and q == 0 and k == 0
                    last = stop and q == KH // KQ - 1 and k == KQ - 1
                    nc.tensor.matmul(
                        ps,
                        lhsT=wt[:, k, g * P:(g + 1) * P],
                        rhs=rt[:, k, ns],
                        start=first, stop=last)

        def eltwise(l, n, psd):
            """psd: dict g -> psum tile [P, NH] holding complete gate
            pre-activations (minus bias for l>0, which lives in acc[l])."""
            ns = slice(n * NH, (n + 1) * NH)
            if l > 0:
                for g in range(4):
                    nc.vector.tensor_tensor(
                        out=psd[g], in0=psd[g],
                        in1=acc[l][:, g, ns], op=ADD)
            b0 = (lambda g: bcol(l, g)) if l == 0 else (lambda g: 0.0)
            it = ep.tile([P, NH], f32, tag="it")
            nc.scalar.activation(it[:], psd[0], AF.Sigmoid, bias=b0(0))
            ft = ep.tile([P, NH], f32, tag="ft")
            nc.scalar.activation(ft[:], psd[1], AF.Sigmoid, bias=b0(1))
            gt = ep.tile([P, NH], f32, tag="gt")
            nc.scalar.activation(gt[:], psd[2], AF.Tanh, bias=b0(2))
            ot = ep.tile([P, NH], f32, tag="ot")
            nc.scalar.activation(ot[:], psd[3], AF.Sigmoid, bias=b0(3))
            cs = ep.tile([P, NH], f32, tag="cs")
            nc.gpsimd.dma_start(out=cs[:], in_=cellT[l, :, ns])
            nc.vector.tensor_tensor(out=ft[:], in0=ft[:], in1=cs[:], op=MULT)
            nc.vector.tensor_tensor(out=it[:], in0=it[:], in1=gt[:], op=MULT)
            nc.vector.tensor_tensor(out=ft[:], in0=ft[:], in1=it[:], op=ADD)
            nc.scalar.dma_start(cT_out[l, :, ns], ft[:])  # c_new write-out
            tc_t = ep.tile([P, NH], f32, tag="gt")
            nc.scalar.activation(tc_t[:], ft[:], AF.Tanh)
            nc.vector.tensor_tensor(out=h_sb[l][:, ns], in0=ot[:],
                                    in1=tc_t[:], op=MULT)
            nc.scalar.dma_start(hT_out[l, :, ns], h_sb[l][:, ns])
            if l < 2:
                # f32 -> bf16/f32r cast DMA must go via gpsimd
                nc.gpsimd.dma_start(out=ag_in[l][:, ns],
                                    in_=h_sb[l][:, ns])

        def trigger_ag(l):
            nc.gpsimd.collective_compute(
                "AllGather", mybir.AluOpType.bypass, replica_groups=RG,
                ins=[ag_in[l].opt()], outs=[ag_out[l].opt()])

        def whh_to_acc(l, wqs, rqs, n):
            ns = slice(n * NH, (n + 1) * NH)
            for g in range(4):
                ps = psg.tile([P, NH], f32, tag="g")
                chain(ps, wqs, rqs, g, n, start=True, stop=True)
                nc.vector.tensor_tensor(
                    out=acc[l][:, g, ns], in0=ps,
                    in1=bcol(l, g).to_broadcast([P, NH]), op=ADD)

        # ---------- layer 0 ----------
        # interleave weight/rhs quarters so the first chain can start after
        # ~0.75MB of DMA
        w0, r0 = [], []
        for q in range(KH // KQ):
            t = wp.tile([P, KQ, GH], MM_DT, tag="w")
            nc.sync.dma_start(t[:], whhT[0][:, q * KQ:(q + 1) * KQ, :])
            w0.append(t)
            t = rp.tile([P, KQ, B], MM_DT, tag="r")
            nc.sync.dma_start(t[:], hiddenT[0][:, q * KQ:(q + 1) * KQ, :])
            r0.append(t)
        w1 = load_w(whhT[1])  # prefetch
        r1 = load_rhs_hidden(1)
        ps00 = {}
        for g in range(4):
            ps = psg.tile([P, NH], f32, tag="g")
            chain(ps, w0, r0, g, 0, start=True, stop=False)
            ps00[g] = ps
        # emb transposes (PE) -> inp0 feature-major; PE has dense work above
        # while the gather finishes.
        for c in range(8):
            pt = pstp.tile([P, P], f32, tag="tp")
            nc.tensor.transpose(pt[0:EMB, :], emb_raw[:, c, :], ident[:])
            nc.vector.tensor_copy(out=inp0[P - EMB:P, c * P:(c + 1) * P],
                                  in_=pt[0:EMB, :])
        for g in range(4):
            nc.tensor.matmul(ps00[g], lhsT=wih0_sb[:, g * P:(g + 1) * P],
                             rhs=inp0[:, 0:NH], start=False, stop=True)
        eltwise(0, 0, ps00)
        ps01 = {}
        for g in range(4):
            ps = psg.tile([P, NH], f32, tag="g")
            chain(ps, w0, r0, g, 1, start=True, stop=False)
            nc.tensor.matmul(ps, lhsT=wih0_sb[:, g * P:(g + 1) * P],
                             rhs=inp0[:, NH:B], start=False, stop=True)
            ps01[g] = ps
        eltwise(0, 1, ps01)
        trigger_ag(0)

        # ---------- pass A layer 1 (w_hh @ hidden_old -> acc) ----------
        for n in range(2):
            whh_to_acc(1, w1, r1, n)

        # head partials accumulate into hacc as each layer's h completes
        hacc = statep.tile([2, B], f32, tag="hacc")

        def head_partial(l):
            for n in range(2):
                ns = slice(n * NH, (n + 1) * NH)
                ph = pshp.tile([2, NH], f32, tag="hd")
                nc.tensor.matmul(ph, lhsT=whead_sb[:, l],
                                 rhs=h_sb[l][:, ns], start=True, stop=True)
                if l == 0:
                    nc.vector.tensor_copy(out=hacc[:, ns], in_=ph)
                else:
                    nc.vector.tensor_tensor(out=hacc[:, ns], in0=hacc[:, ns],
                                            in1=ph, op=ADD)

        # ---------- pass A layer 2 (fills the AG0 wait window) ----------
        w2 = load_w(whhT[2])
        r2 = load_rhs_hidden(2)
        whh_to_acc(2, w2, r2, 0)
        head_partial(0)
        whh_to_acc(2, w2, r2, 1)

        # ---------- pass B layer 1: w_ih @ h0 (gathered) ----------
        wi1 = load_w(wihT[0])
        ri0 = load_rhs_ag(0)
        for n in range(2):
            psd = {}
            for g in range(4):
                ps = psg.tile([P, NH], f32, tag="g")
                chain8(ps, wi1, ri0, g, n, start=True, stop=True)
                psd[g] = ps
            eltwise(1, n, psd)
        trigger_ag(1)
        head_partial(1)

        # ---------- pass B layer 2 ----------
        wi2 = load_w(wihT[1])
        ri1 = load_rhs_ag(1)
        for n in range(2):
            psd = {}
            for g in range(4):
                ps = psg.tile([P, NH], f32, tag="g")
                chain8(ps, wi2, ri1, g, n, start=True, stop=True)
                psd[g] = ps
            eltwise(2, n, psd)
        head_partial(2)

        # ---------- head AllReduce + mu / softplus-sigma ----------
        nc.sync.dma_start(ar_in, hacc[:])
        nc.gpsimd.collective_compute(
            "AllReduce", ADD, replica_groups=RG,
            ins=[ar_in.opt()], outs=[ar_out.opt()])
        mu_sb = statep.tile([1, B], f32, tag="mu")
        nc.scalar.dma_start(mu_sb[:], ar_out[0:1, :])
        ps_sb = statep.tile([1, B], f32, tag="ps")
        nc.sync.dma_start(ps_sb[:], ar_out[1:2, :])
        # mu on DVE (runs parallel to the sigma ACT chain)
        nc.vector.tensor_scalar_add(out=mu_sb[:], in0=mu_sb[:],
                                    scalar1=bh_sb[0:1, 0:1])
        # softplus(x) = ln(1 + exp(x)); x = ps + b_ps is O(1) here so the
        # unstabilized form is exact enough (and sim-supported).
        nc.scalar.activation(ps_sb[:], ps_sb[:], AF.Exp, bias=bh_sb[:, 1:2])
        nc.vector.tensor_scalar_add(out=ps_sb[:], in0=ps_sb[:], scalar1=1.0)
        nc.scalar.activation(ps_sb[:], ps_sb[:], AF.Ln)
        nc.scalar.dma_start(head_out[0:1, :], mu_sb[:])
        nc.sync.dma_start(head_out[1:2, :], ps_sb[:])

    nc.compile()
    return nc


def _shuffle_w(wT):
    """[K, M] -> [P, KH, M] with [p, k, m] = wT[k*128+p, m]."""
    K, M = wT.shape
    kh = K // P
    return np.ascontiguousarray(
        wT.reshape(kh, P, M).transpose(1, 0, 2)).astype(MM_NP)


def shard_inputs(inputs: dict) -> list[dict]:
    """Host-side layout prep + per-core slicing. Data movement only."""
    x = np.asarray(inputs["x"], np.float32)
    idx = np.asarray(inputs["idx"])
    hidden = np.asarray(inputs["hidden"], np.float32)
    cell = np.asarray(inputs["cell"], np.float32)
    emb = np.ascontiguousarray(np.asarray(inputs["emb"], np.float32))
    w_ih0 = np.asarray(inputs["w_ih0"], np.float32)
    w_hh0 = np.asarray(inputs["w_hh0"], np.float32)
    w_ih_rest = np.asarray(inputs["w_ih_rest"], np.float32)
    w_hh_rest = np.asarray(inputs["w_hh_rest"], np.float32)
    b_ih = np.asarray(inputs["b_ih"], np.float32)
    b_hh = np.asarray(inputs["b_hh"], np.float32)
    w_mu = np.asarray(inputs["w_mu"], np.float32)
    b_mu = np.asarray(inputs["b_mu"], np.float32)
    w_ps = np.asarray(inputs["w_ps"], np.float32)
    b_ps = np.asarray(inputs["b_ps"], np.float32)

    xT = np.ascontiguousarray(x[0].T)  # [5, B]
    idx32 = np.ascontiguousarray(
        idx[0].astype(np.int32).reshape(8, P).T)  # [P, 8]
    # hidden [L, B, H] -> [L, P, KH, B] with [l, p, k, b] = hidden[l, b, kP+p]
    hiddenT = np.ascontiguousarray(
        hidden.transpose(0, 2, 1).reshape(L, KH, P, B).transpose(0, 2, 1, 3)
    ).astype(MM_NP)
    b = b_ih + b_hh  # [L, 4H]
    w_ih_all = [w_ih0] + [w_ih_rest[i] for i in range(L - 1)]
    w_hh_all = [w_hh0] + [w_hh_rest[i] for i in range(L - 1)]
    bhead = np.array([[float(b_mu[0]), float(b_ps[0])]], np.float32)

    in_maps = []
    for m in range(NCORES):
        fs = slice(m * P, (m + 1) * P)
        rows = np.concatenate(
            [np.arange(g * H + m * P, g * H + m * P + P) for g in range(4)])
        # w_ih0 rows -> [K, 512]: x-part at K 0:5, emb-part at K 64:128
        w0 = np.zeros((P, GH), np.float32)
        w0[:1 + COV] = w_ih0[rows, :1 + COV].T
        w0[P - EMB:] = w_ih0[rows, 1 + COV:].T
        wihT = np.stack([_shuffle_w(w_ih_all[l][rows].T) for l in (1, 2)])
        whhT = np.stack([_shuffle_w(w_hh_all[l][rows].T) for l in range(L)])
        # bias [P, 12]: col l*4+g = b[l, g*H + feature]
        biasv = np.empty((P, 12), np.float32)
        for l in range(L):
            for g in range(4):
                biasv[:, l * 4 + g] = b[l, g * H + m * P:g * H + m * P + P]
        # head weights [P, L, 2]: [j, l, 0] = w_mu[(m*P+j)*3 + l]
        whead = np.empty((P, L, 2), np.float32)
        for l in range(L):
            whead[:, l, 0] = w_mu[0, (np.arange(m * P, m * P + P)) * L + l]
            whead[:, l, 1] = w_ps[0, (np.arange(m * P, m * P + P)) * L + l]
        in_maps.append({
            "xT": xT,
            "idx32": idx32,
            "emb": emb,
            "hiddenT": hiddenT,
            "cellT": np.ascontiguousarray(cell[:, :, fs].transpose(0, 2, 1)),
            "wih0T": w0.astype(MM_NP),
            "wihT": wihT,
            "whhT": whhT,
            "biasv": biasv,
            "whead": whead,
            "bhead": bhead,
        })
    return in_maps


def assemble_outputs(results: list[dict]):
    hT = np.stack([results[m]["hT_out"] for m in range(NCORES)])  # [M,L,P,B]
    cT = np.stack([results[m]["cT_out"] for m in range(NCORES)])
    # [M, L, P, B] -> [L, B, M*P]
    h_stack = np.ascontiguousarray(hT.transpose(1, 3, 0, 2).reshape(L, B, H))
    c_stack = np.ascontiguousarray(cT.transpose(1, 3, 0, 2).reshape(L, B, H))
    head = results[0]["head_out"]  # [2, B]
    mu = np.ascontiguousarray(head[0])
    sigma = np.ascontiguousarray(head[1])
    return mu, sigma, h_stack, c_stack


_PROGRAM_CACHE = {}


def _get_program() -> bass.Bass:
    if "nc" not in _PROGRAM_CACHE:
        _PROGRAM_CACHE["nc"] = build_program()
    return _PROGRAM_CACHE["nc"]


def kernel(**inputs):
    nc = _get_program()
    in_maps = shard_inputs(inputs)
    res = run_bass_kernel_spmd(nc, in_maps, core_ids=list(range(NCORES)))
    return assemble_outputs(res.results)


# revision 41
# speedup vs baseline: 1.0815x; 1.0815x over previous
"""DeepAR single-step LSTM (3 layers, B=1024, H=1024) on 8 trn2 NeuronCores.

Sharding: model-parallel over the hidden-feature axis. Core m owns features
Fm = [128m, 128m+128) of every gate block in every layer, i.e. rows
{g*1024 + Fm : g in 0..3} of each [4096, K] weight. Everything on device is
feature-major ([features, batch]), so no on-device transposes of activations
are ever needed:

  - gates_m.T [512, B] = Wm.T-chunks (lhsT, [K=128,128]) x state.T chunks
    (rhs, [K=128, 512]), accumulated over K in PSUM.
  - Matmul inputs (weights, old/new hidden states) are bf16 (or fp32r);
    PSUM accumulation, the LSTM cell elementwise math, cell state, biases
    and all outputs stay fp32.
  - LSTM cell elementwise on [128, B] feature-major tiles (ACT + DVE).
  - new h slices are AllGather'd per layer so layers 1/2 input matmuls can
    run; the layer-2 w_hh pass is interleaved to cover AG latency.
  - head (mu/softplus-sigma) = tiny PE matmuls + AllReduce of [2, B]
    partials.

Host side only reshapes/transposes/slices inputs (weight layout prep,
dtype casts) and concatenates outputs; all model math runs on device.
"""

import numpy as np
import ml_dtypes

import concourse.bacc as bacc
import concourse.bass as bass
import concourse.mybir as mybir
import concourse.tile as tile
from concourse.bass import IndirectOffsetOnAxis
from concourse.bass_utils import run_bass_kernel_spmd
from concourse.masks import make_identity

P = 128
B = 1024
H = 1024
L = 3
NCORES = 8
EMB = 64
COV = 4
IN_DIM = 1 + COV + EMB  # 69
NUM_CLASSES = 32768
GH = 4 * P  # gate rows per core (512)
NH = 512  # matmul free-dim chunk (batch half)
KH = H // P  # 8 K-chunks over the hidden dim
KHH = KH // 2

f32 = mybir.dt.float32
i32 = mybir.dt.int32
ADD = mybir.AluOpType.add
MULT = mybir.AluOpType.mult
AF = mybir.ActivationFunctionType

# matmul-input dtype: bfloat16 (full PE rate, FWL weight loads) or float32r
# (single-pass fp32, ~2x slower, higher precision)
MM_DT = mybir.dt.bfloat16
MM_NP = ml_dtypes.bfloat16


def build_program() -> bass.Bass:
    nc = bacc.Bacc("TRN2", target_bir_lowering=False, debug=False,
                   num_devices=NCORES)
    RG = [list(range(NCORES))]

    # ------------- DRAM I/O (per-core shapes) -------------
    # pre-shuffled on host so every DMA is per-partition contiguous:
    #   w*(T): [P, KH, GH]  with [p, k, m] = w[m_row, k*128+p]
    #   hiddenT: [L, P, KH, B] with [l, p, k, b] = hidden[l, b, k*128+p]
    xT = nc.dram_tensor("xT", [1 + COV, B], f32, kind="ExternalInput").ap()
    idx32 = nc.dram_tensor("idx32", [P, 8], i32, kind="ExternalInput").ap()
    emb = nc.dram_tensor("emb", [NUM_CLASSES, EMB], f32,
                         kind="ExternalInput").ap()
    hiddenT = nc.dram_tensor("hiddenT", [L, P, KH, B], MM_DT,
                             kind="ExternalInput").ap()
    cellT = nc.dram_tensor("cellT", [L, P, B], f32, kind="ExternalInput").ap()
    wih0T = nc.dram_tensor("wih0T", [P, GH], MM_DT, kind="ExternalInput").ap()
    wihT = nc.dram_tensor("wihT", [L - 1, P, KH, GH], MM_DT,
                          kind="ExternalInput").ap()
    whhT = nc.dram_tensor("whhT", [L, P, KH, GH], MM_DT,
                          kind="ExternalInput").ap()
    biasv = nc.dram_tensor("biasv", [P, 12], f32, kind="ExternalInput").ap()
    whead = nc.dram_tensor("whead", [P, L, 2], f32, kind="ExternalInput").ap()
    bhead = nc.dram_tensor("bhead", [1, 2], f32, kind="ExternalInput").ap()

    hT_out = nc.dram_tensor("hT_out", [L, P, B], f32,
                            kind="ExternalOutput").ap()
    cT_out = nc.dram_tensor("cT_out", [L, P, B], f32,
                            kind="ExternalOutput").ap()
    head_out = nc.dram_tensor("head_out", [2, B], f32,
                              kind="ExternalOutput").ap()

    # ------------- internal DRAM (collective bounce) -------------
    ag_in = [nc.dram_tensor(f"ag_in_{l}", [P, B], MM_DT).ap()
             for l in range(2)]
    ag_out = [nc.dram_tensor(f"ag_out_{l}", [H, B], MM_DT,
                             addr_space="Shared").ap()
              for l in range(2)]
    ar_in = nc.dram_tensor("ar_in", [2, B], f32).ap()
    ar_out = nc.dram_tensor("ar_out", [2, B], f32, addr_space="Shared").ap()

    with tile.TileContext(nc) as tc, \
            tc.tile_pool(name="const", bufs=1) as constp, \
            tc.tile_pool(name="state", bufs=1) as statep, \
            tc.tile_pool(name="wp", bufs=10) as wp, \
            tc.tile_pool(name="rp", bufs=10) as rp, \
            tc.tile_pool(name="ep", bufs=2) as ep, \
            tc.tile_pool(name="psg", bufs=5, space="PSUM") as psg, \
            tc.tile_pool(name="pst", bufs=2, space="PSUM") as pstp, \
            tc.tile_pool(name="psh", bufs=1, space="PSUM") as pshp:

        # ---------- constants / small inputs ----------
        # latency-class loads go on the scalar HWDGE ring; the sync ring is
        # reserved for the big streaming loads so the first matmuls start
        # as early as possible.
        idx_sb = constp.tile([P, 8], i32, tag="idx")
        nc.scalar.dma_start(idx_sb[:], idx32)
        bias_sb = constp.tile([P, 12], f32, tag="bias")
        nc.scalar.dma_start(bias_sb[:], biasv)
        whead_sb = constp.tile([P, L, 2], f32, tag="whead")
        nc.scalar.dma_start(whead_sb[:], whead)
        bh_sb = constp.tile([1, 2], f32, tag="bh")
        nc.scalar.dma_start(bh_sb[:], bhead)
        ident = constp.tile([P, P], f32, tag="ident")
        make_identity(nc, ident[:])
        wih0_sb = constp.tile([P, GH], MM_DT, tag="wih0")
        nc.scalar.dma_start(wih0_sb[:], wih0T)
        # preload ACT function tables off the critical path
        warm = constp.tile([1, 4], f32, tag="actwarm")
        nc.vector.memset(warm[:], 1.0)
        for fn in (AF.Sigmoid, AF.Tanh):
            nc.scalar.activation(warm[:], warm[:], fn)

        # layer-0 input, feature-major [K=128, B]: x at partitions 0:5,
        # emb at partitions 64:128 (engine writes need 32-aligned starts),
        # rest zero; wih0T host layout matches.
        inp0 = constp.tile([P, B], MM_DT, tag="inp0")
        nc.any.memzero(inp0[:])
        nc.gpsimd.dma_start(out=inp0[0:1 + COV, :], in_=xT)  # cast dma

        # one gather per 128-batch tile: HW indirect DMA honors exactly one
        # offset per partition ([P, 1] offset AP)
        emb_raw = constp.tile([P, 8, EMB], f32, tag="embraw")
        for c in range(8):
            nc.gpsimd.indirect_dma_start(
                out=emb_raw[:, c, :], out_offset=None, in_=emb,
                in_offset=IndirectOffsetOnAxis(ap=idx_sb[:, c:c + 1], axis=0))

        # persistent state
        h_sb = [statep.tile([P, B], f32, tag=f"h{l}", name=f"h{l}")
                for l in range(L)]
        acc = {l: statep.tile([P, 4, B], f32, tag=f"acc{l}", name=f"acc{l}")
               for l in (1, 2)}

        def bcol(l, g):
            return bias_sb[:, l * 4 + g:l * 4 + g + 1]

        KQ = 2  # k-chunks per streamed quarter-tile

        def load_w(dram3d):
            # dram3d: [P, KH, GH] -> four [P, KQ, GH] tiles
            qs = []
            for q in range(KH // KQ):
                t = wp.tile([P, KQ, GH], MM_DT, tag="w")
                nc.sync.dma_start(t[:], dram3d[:, q * KQ:(q + 1) * KQ, :])
                qs.append(t)
            return qs

        def load_rhs_hidden(l):
            # hiddenT[l]: [P, KH, B] -> four [P, KQ, B] tiles (full batch)
            qs = []
            for q in range(KH // KQ):
                t = rp.tile([P, KQ, B], MM_DT, tag="r")
                nc.sync.dma_start(
                    t[:], hiddenT[l][:, q * KQ:(q + 1) * KQ, :])
                qs.append(t)
            return qs

        def load_rhs_ag(l):
            # ag_out[l]: [H, B] rank-block rows -> [P, 1, B] tiles; eighth
            # granularity so pass-B starts ~1us after the AG completes
            r = ag_out[l].rearrange("(ko p) b -> p ko b", p=P)
            es = []
            for q in range(KH):
                t = rp.tile([P, 1, B], MM_DT, tag="re")
                nc.sync.dma_start(t[:], r[:, q:q + 1, :])
                es.append(t)
            return es

        def chain8(ps, wqs, r8, g, n, start, stop):
            # chain variant for eighth-granularity rhs tiles
            ns = slice(n * NH, (n + 1) * NH)
            for kk in range(KH):
                nc.tensor.matmul(
                    ps,
                    lhsT=wqs[kk // KQ][:, kk % KQ, g * P:(g + 1) * P],
                    rhs=r8[kk][:, 0, ns],
                    start=start and kk == 0, stop=stop and kk == KH - 1)

        def chain(ps, wqs, rqs, g, n, start, stop):
            ns = slice(n * NH, (n + 1) * NH)
            for q in range(KH // KQ):
                wt, rt = wqs[q], rqs[q]
                for k in range(KQ):
                    first = start and q == 0 and k == 0
                    last = stop and q == KH // KQ - 1 and k == KQ - 1
                    nc.tensor.matmul(
                        ps,
                        lhsT=wt[:, k, g * P:(g + 1) * P],
                        rhs=rt[:, k, ns],
                        start=first, stop=last)

        def eltwise(l, n, psd):
            """psd: dict g -> psum tile [P, NH] holding complete gate
            pre-activations (minus bias for l>0, which lives in acc[l])."""
            ns = slice(n * NH, (n + 1) * NH)
            if l > 0:
                for g in range(4):
                    nc.vector.tensor_tensor(
                        out=psd[g], in0=psd[g],
                        in1=acc[l][:, g, ns], op=ADD)
            b0 = (lambda g: bcol(l, g)) if l == 0 else (lambda g: 0.0)
            it = ep.tile([P, NH], f32, tag="it")
            nc.scalar.activation(it[:], psd[0], AF.Sigmoid, bias=b0(0))
            ft = ep.tile([P, NH], f32, tag="ft")
            nc.scalar.activation(ft[:], psd[1], AF.Sigmoid, bias=b0(1))
            gt = ep.tile([P, NH], f32, tag="gt")
            nc.scalar.activation(gt[:], psd[2], AF.Tanh, bias=b0(2))
            ot = ep.tile([P, NH], f32, tag="ot")
            nc.scalar.activation(ot[:], psd[3], AF.Sigmoid, bias=b0(3))
            cs = ep.tile([P, NH], f32, tag="cs")
            nc.gpsimd.dma_start(out=cs[:], in_=cellT[l, :, ns])
            nc.vector.tensor_tensor(out=ft[:], in0=ft[:], in1=cs[:], op=MULT)
            nc.vector.tensor_tensor(out=it[:], in0=it[:], in1=gt[:], op=MULT)
            nc.vector.tensor_tensor(out=ft[:], in0=ft[:], in1=it[:], op=ADD)
            nc.scalar.dma_start(cT_out[l, :, ns], ft[:])  # c_new write-out
            tc_t = ep.tile([P, NH], f32, tag="gt")
            nc.scalar.activation(tc_t[:], ft[:], AF.Tanh)
            nc.vector.tensor_tensor(out=h_sb[l][:, ns], in0=ot[:],
                                    in1=tc_t[:], op=MULT)
            nc.scalar.dma_start(hT_out[l, :, ns], h_sb[l][:, ns])
            if l < 2:
                # f32 -> bf16/f32r cast DMA must go via gpsimd
                nc.gpsimd.dma_start(out=ag_in[l][:, ns],
                                    in_=h_sb[l][:, ns])

        def trigger_ag(l):
            nc.gpsimd.collective_compute(
                "AllGather", mybir.AluOpType.bypass, replica_groups=RG,
                ins=[ag_in[l].opt()], outs=[ag_out[l].opt()])

        def whh_to_acc(l, wqs, rqs, n):
            ns = slice(n * NH, (n + 1) * NH)
            for g in range(4):
                ps = psg.tile([P, NH], f32, tag="g")
                chain(ps, wqs, rqs, g, n, start=True, stop=True)
                nc.vector.tensor_tensor(
                    out=acc[l][:, g, ns], in0=ps,
                    in1=bcol(l, g).to_broadcast([P, NH]), op=ADD)

        # ---------- layer 0 ----------
        # interleave weight/rhs quarters so the first chain can start after
        # ~0.75MB of DMA
        w0, r0 = [], []
        for q in range(KH // KQ):
            t = wp.tile([P, KQ, GH], MM_DT, tag="w")
            nc.sync.dma_start(t[:], whhT[0][:, q * KQ:(q + 1) * KQ, :])
            w0.append(t)
            t = rp.tile([P, KQ, B], MM_DT, tag="r")
            nc.sync.dma_start(t[:], hiddenT[0][:, q * KQ:(q + 1) * KQ, :])
            r0.append(t)
        w1 = load_w(whhT[1])  # prefetch
        r1 = load_rhs_hidden(1)
        ps00 = {}
        for g in range(4):
            ps = psg.tile([P, NH], f32, tag="g")
            chain(ps, w0, r0, g, 0, start=True, stop=False)
            ps00[g] = ps
        # emb transposes (PE) -> inp0 feature-major; PE has dense work above
        # while the gather finishes.
        for c in range(8):
            pt = pstp.tile([P, P], f32, tag="tp")
            nc.tensor.transpose(pt[0:EMB, :], emb_raw[:, c, :], ident[:])
            nc.vector.tensor_copy(out=inp0[P - EMB:P, c * P:(c + 1) * P],
                                  in_=pt[0:EMB, :])
        for g in range(4):
            nc.tensor.matmul(ps00[g], lhsT=wih0_sb[:, g * P:(g + 1) * P],
                             rhs=inp0[:, 0:NH], start=False, stop=True)
        eltwise(0, 0, ps00)
        ps01 = {}
        for g in range(4):
            ps = psg.tile([P, NH], f32, tag="g")
            chain(ps, w0, r0, g, 1, start=True, stop=False)
            nc.tensor.matmul(ps, lhsT=wih0_sb[:, g * P:(g + 1) * P],
                             rhs=inp0[:, NH:B], start=False, stop=True)
            ps01[g] = ps
        eltwise(0, 1, ps01)
        trigger_ag(0)

        # ---------- pass A layer 1 (w_hh @ hidden_old -> acc) ----------
        for n in range(2):
            whh_to_acc(1, w1, r1, n)

        # head partials accumulate into hacc as each layer's h completes
        hacc = statep.tile([2, B], f32, tag="hacc")

        def head_partial(l):
            for n in range(2):
                ns = slice(n * NH, (n + 1) * NH)
                ph = pshp.tile([2, NH], f32, tag="hd")
                nc.tensor.matmul(ph, lhsT=whead_sb[:, l],
                                 rhs=h_sb[l][:, ns], start=True, stop=True)
                if l == 0:
                    nc.vector.tensor_copy(out=hacc[:, ns], in_=ph)
                else:
                    nc.vector.tensor_tensor(out=hacc[:, ns], in0=hacc[:, ns],
                                            in1=ph, op=ADD)

        # ---------- pass A layer 2 first half (fills the AG0 window) ----
        w2 = load_w(whhT[2])
        r2 = load_rhs_hidden(2)
        whh_to_acc(2, w2, r2, 0)
        head_partial(0)

        # ---------- pass B layer 1: w_ih @ h0 (gathered) ----------
        wi1 = load_w(wihT[0])
        ri0 = load_rhs_ag(0)
        for n in range(2):
            psd = {}
            for g in range(4):
                ps = psg.tile([P, NH], f32, tag="g")
                chain8(ps, wi1, ri0, g, n, start=True, stop=True)
                psd[g] = ps
            eltwise(1, n, psd)
        trigger_ag(1)
        # fills the AG1 wait window
        whh_to_acc(2, w2, r2, 1)
        head_partial(1)

        # ---------- pass B layer 2 ----------
        wi2 = load_w(wihT[1])
        ri1 = load_rhs_ag(1)
        for n in range(2):
            psd = {}
            for g in range(4):
                ps = psg.tile([P, NH], f32, tag="g")
                chain8(ps, wi2, ri1, g, n, start=True, stop=True)
                psd[g] = ps
            eltwise(2, n, psd)
        head_partial(2)

        # ---------- head AllReduce + mu / softplus-sigma ----------
        nc.sync.dma_start(ar_in, hacc[:])
        nc.gpsimd.collective_compute(
            "AllReduce", ADD, replica_groups=RG,
            ins=[ar_in.opt()], outs=[ar_out.opt()])
        mu_sb = statep.tile([1, B], f32, tag="mu")
        nc.scalar.dma_start(mu_sb[:], ar_out[0:1, :])
        ps_sb = statep.tile([1, B], f32, tag="ps")
        nc.sync.dma_start(ps_sb[:], ar_out[1:2, :])
        # mu on DVE (runs parallel to the sigma ACT chain)
        nc.vector.tensor_scalar_add(out=mu_sb[:], in0=mu_sb[:],
                                    scalar1=bh_sb[0:1, 0:1])
        # softplus(x) = ln(1 + exp(x)); x = ps + b_ps is O(1) here so the
        # unstabilized form is exact enough (and sim-supported).
        nc.scalar.activation(ps_sb[:], ps_sb[:], AF.Exp, bias=bh_sb[:, 1:2])
        nc.vector.tensor_scalar_add(out=ps_sb[:], in0=ps_sb[:], scalar1=1.0)
        nc.scalar.activation(ps_sb[:], ps_sb[:], AF.Ln)
        nc.scalar.dma_start(head_out[0:1, :], mu_sb[:])
        nc.sync.dma_start(head_out[1:2, :], ps_sb[:])

    nc.compile()
    return nc


def _shuffle_w(wT):
    """[K, M] -> [P, KH, M] with [p, k, m] = wT[k*128+p, m]."""
    K, M = wT.shape
    kh = K // P
    return np.ascontiguousarray(
        wT.reshape(kh, P, M).transpose(1, 0, 2)).astype(MM_NP)


def shard_inputs(inputs: dict) -> list[dict]:
    """Host-side layout prep + per-core slicing. Data movement only."""
    x = np.asarray(inputs["x"], np.float32)
    idx = np.asarray(inputs["idx"])
    hidden = np.asarray(inputs["hidden"], np.float32)
    cell = np.asarray(inputs["cell"], np.float32)
    emb = np.ascontiguousarray(np.asarray(inputs["emb"], np.float32))
    w_ih0 = np.asarray(inputs["w_ih0"], np.float32)
    w_hh0 = np.asarray(inputs["w_hh0"], np.float32)
    w_ih_rest = np.asarray(inputs["w_ih_rest"], np.float32)
    w_hh_rest = np.asarray(inputs["w_hh_rest"], np.float32)
    b_ih = np.asarray(inputs["b_ih"], np.float32)
    b_hh = np.asarray(inputs["b_hh"], np.float32)
    w_mu = np.asarray(inputs["w_mu"], np.float32)
    b_mu = np.asarray(inputs["b_mu"], np.float32)
    w_ps = np.asarray(inputs["w_ps"], np.float32)
    b_ps = np.asarray(inputs["b_ps"], np.float32)

    xT = np.ascontiguousarray(x[0].T)  # [5, B]
    idx32 = np.ascontiguousarray(
        idx[0].astype(np.int32).reshape(8, P).T)  # [P, 8]
    # hidden [L, B, H] -> [L, P, KH, B] with [l, p, k, b] = hidden[l, b, kP+p]
    hiddenT = np.ascontiguousarray(
        hidden.transpose(0, 2, 1).reshape(L, KH, P, B).transpose(0, 2, 1, 3)
    ).astype(MM_NP)
    b = b_ih + b_hh  # [L, 4H]
    w_ih_all = [w_ih0] + [w_ih_rest[i] for i in range(L - 1)]
    w_hh_all = [w_hh0] + [w_hh_rest[i] for i in range(L - 1)]
    bhead = np.array([[float(b_mu[0]), float(b_ps[0])]], np.float32)

    in_maps = []
    for m in range(NCORES):
        fs = slice(m * P, (m + 1) * P)
        rows = np.concatenate(
            [np.arange(g * H + m * P, g * H + m * P + P) for g in range(4)])
        # w_ih0 rows -> [K, 512]: x-part at K 0:5, emb-part at K 64:128
        w0 = np.zeros((P, GH), np.float32)
        w0[:1 + COV] = w_ih0[rows, :1 + COV].T
        w0[P - EMB:] = w_ih0[rows, 1 + COV:].T
        wihT = np.stack([_shuffle_w(w_ih_all[l][rows].T) for l in (1, 2)])
        whhT = np.stack([_shuffle_w(w_hh_all[l][rows].T) for l in range(L)])
        # bias [P, 12]: col l*4+g = b[l, g*H + feature]
        biasv = np.empty((P, 12), np.float32)
        for l in range(L):
            for g in range(4):
                biasv[:, l * 4 + g] = b[l, g * H + m * P:g * H + m * P + P]
        # head weights [P, L, 2]: [j, l, 0] = w_mu[(m*P+j)*3 + l]
        whead = np.empty((P, L, 2), np.float32)
        for l in range(L):
            whead[:, l, 0] = w_mu[0, (np.arange(m * P, m * P + P)) * L + l]
            whead[:, l, 1] = w_ps[0, (np.arange(m * P, m * P + P)) * L + l]
        in_maps.append({
            "xT": xT,
            "idx32": idx32,
            "emb": emb,
            "hiddenT": hiddenT,
            "cellT": np.ascontiguousarray(cell[:, :, fs].transpose(0, 2, 1)),
            "wih0T": w0.astype(MM_NP),
            "wihT": wihT,
            "whhT": whhT,
            "biasv": biasv,
            "whead": whead,
            "bhead": bhead,
        })
    return in_maps


def assemble_outputs(results: list[dict]):
    hT = np.stack([results[m]["hT_out"] for m in range(NCORES)])  # [M,L,P,B]
    cT = np.stack([results[m]["cT_out"] for m in range(NCORES)])
    # [M, L, P, B] -> [L, B, M*P]
    h_stack = np.ascontiguousarray(hT.transpose(1, 3, 0, 2).reshape(L, B, H))
    c_stack = np.ascontiguousarray(cT.transpose(1, 3, 0, 2).reshape(L, B, H))
    head = results[0]["head_out"]  # [2, B]
    mu = np.ascontiguousarray(head[0])
    sigma = np.ascontiguousarray(head[1])
    return mu, sigma, h_stack, c_stack


_PROGRAM_CACHE = {}


def _get_program() -> bass.Bass:
    if "nc" not in _PROGRAM_CACHE:
        _PROGRAM_CACHE["nc"] = build_program()
    return _PROGRAM_CACHE["nc"]


def kernel(**inputs):
    nc = _get_program()
    in_maps = shard_inputs(inputs)
    res = run_bass_kernel_spmd(nc, in_maps, core_ids=list(range(NCORES)))
    return assemble_outputs(res.results)
